# revision 11
# baseline (speedup 1.0000x reference)
"""Tensor-parallel int8-dequant linear for Trainium2 (8 NeuronCores).

out[m, n] = (sum_k act[m, k] * w[k, n]) * scale[n],  act fp16 [32, 8192],
w int8 [8192, 28672], scale fp16 [1, 28672], out fp16 [32, 28672].

Sharding: weight/scale column-sharded over 8 cores (3584 cols each), act
replicated; each core computes its output slice independently (no
collectives), host concatenates.

Per-core kernel: stream int8 weight k-tiles [128, 3584] from HBM
(memory-bound roofline), dequantize on-chip by viewing bytes as int16 and
extracting the two byte classes (DVE bitwise pass + fp cast pass, exact;
ACT engine takes part of the odd-byte class via strided int8 copies), then
4x column-tiled fp16 matmuls (M=32 -> 4 concurrent 32-col PE groups, one
n-quarter each) accumulate into 2 PSUM banks. Epilogue multiplies by the
dequant scale and interleaves even/odd columns on-chip.
"""

import sys

sys.path.insert(0, "/opt/trn_rl_repo")
import numpy as np

M, K, N = 32, 8192, 28672
N_CORES = 8
NSH = N // N_CORES      # 3584 cols per core
NHALF = NSH // 2        # 1792 even (or odd) cols
NQ = NHALF // 4         # 448 cols per PE column-group
KT = K // 128           # 64 k-tiles
SPLIT_C = 320           # odd-class cols [0, SPLIT_C) on DVE, rest on ACT

_cache = {}


def _split_multi_waits(nc, mybir):
    """walrus in this toolchain accepts only ONE sync-wait per instruction;
    Tile attaches one wait per pending logical proc. Move all but the last
    wait of each instruction onto single-wait NoOps inserted just before it
    on the same engine (per-engine program order is list order)."""
    n_split = 0
    for fn in nc.m.functions:
        for blk in fn.blocks:
            insts = list(blk.instructions)
            if not any(
                getattr(i, "sync_info", None) is not None
                and i.sync_info.on_wait
                and len(i.sync_info.on_wait) > 1
                for i in insts
            ):
                continue
            new = []
            for inst in insts:
                si = getattr(inst, "sync_info", None)
                if si is not None and si.on_wait and len(si.on_wait) > 1:
                    waits = list(si.on_wait)
                    si.on_wait = waits[-1:]
                    for w in waits[:-1]:
                        nop = mybir.InstNoOp(
                            name=nc.get_next_instruction_name(), ins=[], outs=[]
                        )
                        nop.engine = inst.engine
                        nop.sync_info = mybir.SyncInfo(on_update=[], on_wait=[w])
                        new.append(nop)
                        n_split += 1
                new.append(inst)
            blk.instructions = new
    return n_split


def _build(kt=KT, split_waits=True):
    import concourse.bass as bass
    import concourse.mybir as mybir
    import concourse.tile as tile

    AF = mybir.AluOpType
    f16, f32, i8, i16 = (
        mybir.dt.float16, mybir.dt.float32, mybir.dt.int8, mybir.dt.int16,
    )

    nc = bass.Bass()
    actT_ext = nc.declare_dram_parameter("actT", [128, kt * M], f16, isOutput=False)
    w_ext = nc.declare_dram_parameter("w", [kt * 128, NSH], i8, isOutput=False)
    se_ext = nc.declare_dram_parameter("se", [128, NQ], f16, isOutput=False)
    so_ext = nc.declare_dram_parameter("so", [128, NQ], f16, isOutput=False)
    out_ext = nc.declare_dram_parameter("out", [M, NSH], f16, isOutput=True)

    with (
        tile.TileContext(nc) as tc,
        tc.tile_pool(name="const", bufs=1) as cpool,
        tc.tile_pool(name="w8", bufs=6) as wpool,
        tc.tile_pool(name="cast", bufs=3) as castpool,
        tc.tile_pool(name="psum", bufs=2, space="PSUM") as psum,
    ):
        at = cpool.tile([128, kt, M], f16)
        se = cpool.tile([128, NQ], f16)
        so = cpool.tile([128, NQ], f16)

        ps_e = psum.tile([128, NQ], f32)
        ps_o = psum.tile([128, NQ], f32)
        out_sb = cpool.tile([M, NSH], f16)

        assert kt % 2 == 0
        for kk in range(kt // 2):
            # two k-tiles per iteration: bigger DMA + bigger DVE ops
            w8 = wpool.tile([128, 2, NSH], i8, tag="w8")
            for t in range(2):
                nc.sync.dma_start(
                    out=w8[:, t, :],
                    in_=w_ext[kk * 256 + t * 128:kk * 256 + (t + 1) * 128, :],
                )
            if kk == 0:
                nc.sync.dma_start(
                    out=at[:],
                    in_=actT_ext[:].rearrange("p (kt m) -> p kt m", kt=kt),
                )
            w16 = w8[:].bitcast(i16)  # [128, 2*NHALF]
            w16f = w16.rearrange("p t n -> p (t n)")

            even = castpool.tile([128, 2, NHALF], f16, tag="even")
            odd = castpool.tile([128, 2, NHALF], f16, tag="odd")
            lo16 = castpool.tile([128, 2 * NHALF], i16, tag="lo")
            hi16 = castpool.tile([128, 2, SPLIT_C], i16, tag="hi")

            # even bytes (all cols), exact: ((w16 ^ 0x80) & 0xFF) - 128
            # odd bytes [0, SPLIT_C): (w16 & 0xFF00) * (1/256)
            # odd bytes [SPLIT_C, NHALF) on ACT via strided int8 copy
            cast_parts = [(0, 2)] if kk > 0 else [(0, 1), (1, 2)]
            for (t0, t1) in cast_parts:
                nc.vector.tensor_scalar(
                    out=lo16[:, t0 * NHALF:t1 * NHALF],
                    in0=w16[:, t0:t1, :].rearrange("p t n -> p (t n)"),
                    scalar1=0x80, scalar2=0xFF,
                    op0=AF.bitwise_xor, op1=AF.bitwise_and,
                )
                nc.vector.tensor_scalar(
                    out=even[:, t0:t1, :].rearrange("p t n -> p (t n)"),
                    in0=lo16[:, t0 * NHALF:t1 * NHALF],
                    scalar1=-128.0, scalar2=None, op0=AF.add,
                )
                nc.vector.tensor_scalar(
                    out=hi16[:, t0:t1, :], in0=w16[:, t0:t1, 0:SPLIT_C],
                    scalar1=-256, scalar2=None, op0=AF.bitwise_and,
                )
                nc.vector.tensor_scalar(
                    out=odd[:, t0:t1, 0:SPLIT_C], in0=hi16[:, t0:t1, :],
                    scalar1=1.0 / 256.0, scalar2=None, op0=AF.mult,
                )
                nc.scalar.copy(
                    out=odd[:, t0:t1, SPLIT_C:NHALF],
                    in_=w8[:, t0:t1, 2 * SPLIT_C + 1::2],
                )

            for t in range(2):
                k = kk * 2 + t
                for g in range(4):
                    nc.tensor.matmul(
                        out=ps_e[32 * g:32 * g + 32, :],
                        lhsT=at[:, k, :],
                        rhs=even[:, t, NQ * g:NQ * (g + 1)],
                        start=(k == 0), stop=(k == kt - 1),
                        tile_position=(0, 32 * g),
                        skip_group_check=True,
                    )
                for g in range(4):
                    nc.tensor.matmul(
                        out=ps_o[32 * g:32 * g + 32, :],
                        lhsT=at[:, k, :],
                        rhs=odd[:, t, NQ * g:NQ * (g + 1)],
                        start=(k == 0), stop=(k == kt - 1),
                        tile_position=(0, 32 * g),
                        skip_group_check=True,
                    )

        nc.sync.dma_start(out=se[:], in_=se_ext[:])
        nc.sync.dma_start(out=so[:], in_=so_ext[:])
        # epilogue: out[m, 2*(448q+j)(+1)] = psum[32q+m, j] * scale
        for q in range(4):
            nc.vector.tensor_tensor(
                out_sb[:, 2 * NQ * q:2 * NQ * (q + 1):2],
                ps_e[32 * q:32 * q + 32, :],
                se[32 * q:32 * q + 32, :],
                AF.mult,
            )
            nc.vector.tensor_tensor(
                out_sb[:, 2 * NQ * q + 1:2 * NQ * (q + 1):2],
                ps_o[32 * q:32 * q + 32, :],
                so[32 * q:32 * q + 32, :],
                AF.mult,
            )
        nc.sync.dma_start(out=out_ext[:], in_=out_sb[:])

    if split_waits:
        _split_multi_waits(nc, mybir)
    return nc


def _prep_inputs(act, weight, scale):
    act = np.ascontiguousarray(np.asarray(act, dtype=np.float16))
    weight = np.asarray(weight)
    if weight.dtype != np.int8:
        weight = weight.astype(np.int8)
    scale = np.asarray(scale, dtype=np.float16).reshape(-1)

    # actT packed: [128, KT, M]: actT_packed[p, kt, m] = act[m, kt*128 + p]
    actT = np.ascontiguousarray(
        act.T.reshape(KT, 128, M).transpose(1, 0, 2).reshape(128, KT * M)
    )

    in_maps = []
    for c in range(N_CORES):
        wsh = np.ascontiguousarray(weight[:, c * NSH:(c + 1) * NSH])
        ssh = scale[c * NSH:(c + 1) * NSH]
        se = np.ascontiguousarray(
            np.broadcast_to(
                ssh[0::2].reshape(4, 1, NQ), (4, 32, NQ)
            ).reshape(128, NQ)
        )
        so = np.ascontiguousarray(
            np.broadcast_to(
                ssh[1::2].reshape(4, 1, NQ), (4, 32, NQ)
            ).reshape(128, NQ)
        )
        in_maps.append({"actT": actT, "w": wsh, "se": se, "so": so})
    return in_maps


def _run(in_maps, trace=False, **kwargs):
    from concourse.bass_utils import run_bass_kernel_spmd

    if "nc" not in _cache:
        _cache["nc"] = _build()
    return run_bass_kernel_spmd(
        _cache["nc"], in_maps, core_ids=list(range(N_CORES)), trace=trace, **kwargs
    )


def kernel(act, weight, scale):
    in_maps = _prep_inputs(act, weight, scale)
    res = _run(in_maps)
    out = np.concatenate([res.results[c]["out"] for c in range(N_CORES)], axis=1)
    return out.astype(np.float16)


# revision 12
# speedup vs baseline: 1.0316x; 1.0316x over previous
"""Tensor-parallel int8-dequant linear for Trainium2 (8 NeuronCores).

out[m, n] = (sum_k act[m, k] * w[k, n]) * scale[n],  act fp16 [32, 8192],
w int8 [8192, 28672], scale fp16 [1, 28672], out fp16 [32, 28672].

Sharding: weight/scale column-sharded over 8 cores (3584 cols each), act
replicated; each core computes its output slice independently (no
collectives), host concatenates.

Per-core kernel: stream int8 weight k-tiles [128, 3584] from HBM
(memory-bound roofline), dequantize on-chip by viewing bytes as int16 and
extracting the two byte classes (DVE bitwise pass + fp cast pass, exact;
ACT engine takes part of the odd-byte class via strided int8 copies), then
4x column-tiled fp16 matmuls (M=32 -> 4 concurrent 32-col PE groups, one
n-quarter each) accumulate into 2 PSUM banks. Epilogue multiplies by the
dequant scale and interleaves even/odd columns on-chip.
"""

import sys

sys.path.insert(0, "/opt/trn_rl_repo")
import numpy as np

M, K, N = 32, 8192, 28672
N_CORES = 8
NSH = N // N_CORES      # 3584 cols per core
NHALF = NSH // 2        # 1792 even (or odd) cols
NQ = NHALF // 4         # 448 cols per PE column-group
KT = K // 128           # 64 k-tiles
SPLIT_C = 320           # odd-class cols [0, SPLIT_C) on DVE, rest on ACT

_cache = {}


def _split_multi_waits(nc, mybir):
    """walrus in this toolchain accepts only ONE sync-wait per instruction;
    Tile attaches one wait per pending logical proc. Move all but the last
    wait of each instruction onto single-wait NoOps inserted just before it
    on the same engine (per-engine program order is list order)."""
    n_split = 0
    for fn in nc.m.functions:
        for blk in fn.blocks:
            insts = list(blk.instructions)
            if not any(
                getattr(i, "sync_info", None) is not None
                and i.sync_info.on_wait
                and len(i.sync_info.on_wait) > 1
                for i in insts
            ):
                continue
            new = []
            for inst in insts:
                si = getattr(inst, "sync_info", None)
                if si is not None and si.on_wait and len(si.on_wait) > 1:
                    waits = list(si.on_wait)
                    si.on_wait = waits[-1:]
                    for w in waits[:-1]:
                        nop = mybir.InstNoOp(
                            name=nc.get_next_instruction_name(), ins=[], outs=[]
                        )
                        nop.engine = inst.engine
                        nop.sync_info = mybir.SyncInfo(on_update=[], on_wait=[w])
                        new.append(nop)
                        n_split += 1
                new.append(inst)
            blk.instructions = new
    return n_split


def _build(kt=KT, split_waits=True):
    import concourse.bass as bass
    import concourse.mybir as mybir
    import concourse.tile as tile

    AF = mybir.AluOpType
    f16, f32, i8, i16 = (
        mybir.dt.float16, mybir.dt.float32, mybir.dt.int8, mybir.dt.int16,
    )

    nc = bass.Bass()
    actT_ext = nc.declare_dram_parameter("actT", [128, kt * M], f16, isOutput=False)
    w_ext = nc.declare_dram_parameter("w", [kt * 128, NSH], i8, isOutput=False)
    se_ext = nc.declare_dram_parameter("se", [128, NQ], f16, isOutput=False)
    so_ext = nc.declare_dram_parameter("so", [128, NQ], f16, isOutput=False)
    out_ext = nc.declare_dram_parameter("out", [M, NSH], f16, isOutput=True)

    with (
        tile.TileContext(nc) as tc,
        tc.tile_pool(name="const", bufs=1) as cpool,
        tc.tile_pool(name="w8", bufs=6) as wpool,
        tc.tile_pool(name="cast", bufs=3) as castpool,
        tc.tile_pool(name="psum", bufs=2, space="PSUM") as psum,
    ):
        at = cpool.tile([128, kt, M], f16)
        se = cpool.tile([128, NQ], f16)
        so = cpool.tile([128, NQ], f16)

        ps_e = psum.tile([128, NQ], f32)
        ps_o = psum.tile([128, NQ], f32)
        out_sb = cpool.tile([M, NSH], f16)

        assert kt % 2 == 0
        for kk in range(kt // 2):
            # two k-tiles per iteration: bigger DMA + bigger DVE ops
            w8 = wpool.tile([128, 2, NSH], i8, tag="w8")
            for t in range(2):
                nc.sync.dma_start(
                    out=w8[:, t, :],
                    in_=w_ext[kk * 256 + t * 128:kk * 256 + (t + 1) * 128, :],
                )
            if kk == 0:
                nc.sync.dma_start(
                    out=at[:],
                    in_=actT_ext[:].rearrange("p (kt m) -> p kt m", kt=kt),
                )
            w16 = w8[:].bitcast(i16)  # [128, 2*NHALF]
            w16f = w16.rearrange("p t n -> p (t n)")

            even = castpool.tile([128, 2, NHALF], f16, tag="even")
            odd = castpool.tile([128, 2, NHALF], f16, tag="odd")
            lo16 = castpool.tile([128, 2 * NHALF], i16, tag="lo")
            hi16 = castpool.tile([128, 2, SPLIT_C], i16, tag="hi")

            # even bytes (all cols), exact: ((w16 ^ 0x80) & 0xFF) - 128
            # odd bytes [0, SPLIT_C): (w16 & 0xFF00) * (1/256)
            # odd bytes [SPLIT_C, NHALF) on ACT via strided int8 copy
            cast_parts = [(0, 2)]
            for (t0, t1) in cast_parts:
                nc.vector.tensor_scalar(
                    out=lo16[:, t0 * NHALF:t1 * NHALF],
                    in0=w16[:, t0:t1, :].rearrange("p t n -> p (t n)"),
                    scalar1=0x80, scalar2=0xFF,
                    op0=AF.bitwise_xor, op1=AF.bitwise_and,
                )
                nc.vector.tensor_scalar(
                    out=even[:, t0:t1, :].rearrange("p t n -> p (t n)"),
                    in0=lo16[:, t0 * NHALF:t1 * NHALF],
                    scalar1=-128.0, scalar2=None, op0=AF.add,
                )
                nc.vector.tensor_scalar(
                    out=hi16[:, t0:t1, :], in0=w16[:, t0:t1, 0:SPLIT_C],
                    scalar1=-256, scalar2=None, op0=AF.bitwise_and,
                )
                nc.vector.tensor_scalar(
                    out=odd[:, t0:t1, 0:SPLIT_C], in0=hi16[:, t0:t1, :],
                    scalar1=1.0 / 256.0, scalar2=None, op0=AF.mult,
                )
                nc.scalar.copy(
                    out=odd[:, t0:t1, SPLIT_C:NHALF],
                    in_=w8[:, t0:t1, 2 * SPLIT_C + 1::2],
                )

            for t in range(2):
                k = kk * 2 + t
                for g in range(4):
                    nc.tensor.matmul(
                        out=ps_e[32 * g:32 * g + 32, :],
                        lhsT=at[:, k, :],
                        rhs=even[:, t, NQ * g:NQ * (g + 1)],
                        start=(k == 0), stop=(k == kt - 1),
                        tile_position=(0, 32 * g),
                        skip_group_check=True,
                    )
                for g in range(4):
                    nc.tensor.matmul(
                        out=ps_o[32 * g:32 * g + 32, :],
                        lhsT=at[:, k, :],
                        rhs=odd[:, t, NQ * g:NQ * (g + 1)],
                        start=(k == 0), stop=(k == kt - 1),
                        tile_position=(0, 32 * g),
                        skip_group_check=True,
                    )

        nc.sync.dma_start(out=se[:], in_=se_ext[:])
        nc.sync.dma_start(out=so[:], in_=so_ext[:])
        # epilogue: out[m, 2*(448q+j)(+1)] = psum[32q+m, j] * scale
        for q in range(4):
            nc.vector.tensor_tensor(
                out_sb[:, 2 * NQ * q:2 * NQ * (q + 1):2],
                ps_e[32 * q:32 * q + 32, :],
                se[32 * q:32 * q + 32, :],
                AF.mult,
            )
            nc.vector.tensor_tensor(
                out_sb[:, 2 * NQ * q + 1:2 * NQ * (q + 1):2],
                ps_o[32 * q:32 * q + 32, :],
                so[32 * q:32 * q + 32, :],
                AF.mult,
            )
        nc.sync.dma_start(out=out_ext[:], in_=out_sb[:])

    if split_waits:
        _split_multi_waits(nc, mybir)
    return nc


def _prep_inputs(act, weight, scale):
    act = np.ascontiguousarray(np.asarray(act, dtype=np.float16))
    weight = np.asarray(weight)
    if weight.dtype != np.int8:
        weight = weight.astype(np.int8)
    scale = np.asarray(scale, dtype=np.float16).reshape(-1)

    # actT packed: [128, KT, M]: actT_packed[p, kt, m] = act[m, kt*128 + p]
    actT = np.ascontiguousarray(
        act.T.reshape(KT, 128, M).transpose(1, 0, 2).reshape(128, KT * M)
    )

    in_maps = []
    for c in range(N_CORES):
        wsh = np.ascontiguousarray(weight[:, c * NSH:(c + 1) * NSH])
        ssh = scale[c * NSH:(c + 1) * NSH]
        se = np.ascontiguousarray(
            np.broadcast_to(
                ssh[0::2].reshape(4, 1, NQ), (4, 32, NQ)
            ).reshape(128, NQ)
        )
        so = np.ascontiguousarray(
            np.broadcast_to(
                ssh[1::2].reshape(4, 1, NQ), (4, 32, NQ)
            ).reshape(128, NQ)
        )
        in_maps.append({"actT": actT, "w": wsh, "se": se, "so": so})
    return in_maps


def _run(in_maps, trace=False, **kwargs):
    from concourse.bass_utils import run_bass_kernel_spmd

    if "nc" not in _cache:
        _cache["nc"] = _build()
    return run_bass_kernel_spmd(
        _cache["nc"], in_maps, core_ids=list(range(N_CORES)), trace=trace, **kwargs
    )


def kernel(act, weight, scale):
    in_maps = _prep_inputs(act, weight, scale)
    res = _run(in_maps)
    out = np.concatenate([res.results[c]["out"] for c in range(N_CORES)], axis=1)
    return out.astype(np.float16)
